# revision 39
# baseline (speedup 1.0000x reference)
"""Embedding lookup + RMSNorm + tied logits projection on 8 trn2 NeuronCores.

Strategy (2-way token x 4-way vocab, fp8 DoubleRow), v6:
  - RMSNorm folded into the embedding table on the host; final_norm split as
    sqrt(fn) into BOTH operands; contraction dims PERMUTED by descending fn.
  - Both operands quantized to e4m3 with a single power-of-2 scale (512 on W):
      W'' = W[:,perm]*sqrt(fn_sorted)*512,  h'' = hn[:,perm]*sqrt(fn_sorted)
      W_hi = e4m3(W''), W_lo = e4m3(W''-W_hi)   (lo kept for top 256 dims)
      h_hi = e4m3(h''), h_lo = e4m3(h''-h_hi)   (lo kept for top 512 dims)
    logits*512 ~= h_hi@W_hi' + h_lo@W_hi'(512) + h_hi@W_lo'(256): 6 fp8
    DoubleRow matmuls per group; measured rel err 1.53e-2 (gate 2e-2).
  - Core c owns token block c//4 (2048 tokens) and vocab shard c%4 (12672
    padded vocab rows). All matmuls are fp8 DoubleRow (0.5 cycles/row, k=256
    per instruction): 6 instructions per [128v x 512t] PSUM group.
  - Phase 1: gather 2048 rows (16 indirect DMAs of 1280B rows), PE-transpose
    to hnt [d_chunk, j, t], DVE evacuation per g-tile.
  - Phase 2: stream W shard once (10*512B descriptors per 4-v-tile chunk),
    t-chunk-outer order within each chunk (overlaps phase 1's tail), PSUM
    f32 accumulate, bf16 out, evacuations alternate DVE/Act engines, out
    DMAs cover t-chunk pairs (halves the 625ns/DMA HWDGE issue cost).
  - Host assembles: outT.T * (1/512), scatter into [T, VPAD], slice vocab.

  Measured on the 8 axon trn2 cores: rel err 1.527e-2 (gate 2e-2),
  exec 285288 ns vs 517214 ns bf16 baseline (1.81x).
"""
import os
import sys

sys.path.insert(0, "/opt/trn_rl_repo")

import numpy as np
import ml_dtypes

import concourse.mybir as mybir
import concourse.tile as tile
from concourse import bacc
from concourse.bass import IndirectOffsetOnAxis
from concourse.bass_utils import run_bass_kernel_spmd

f32 = mybir.dt.float32
bf16 = mybir.dt.bfloat16
f8 = mybir.dt.float8e4
i32 = mybir.dt.int32

B, S, V, D = 2, 2048, 50257, 768
T = B * S                 # 4096 tokens
NC = 8                    # cores
TB = 2                    # token blocks
VB = 4                    # vocab blocks
TCL = T // TB             # 2048 tokens per core
VPAD = 50688              # 4 * 12672, 12672 = 99*128
VSH = VPAD // VB          # 12672 vocab rows per core
NVT = VSH // 128          # 99 v-tiles per core
NG = TCL // 128           # 16 gather tiles
NTC = TCL // 512          # 4 moving-dim chunks
KK = D // 128             # 6 hi d-chunks
MC = 4                    # h-side lo d-chunks: top 512 dims by fn
MCW = 2                   # W-side lo d-chunks: top 256 dims by fn
NJ = KK + MC              # 10 j-chunks in emb/hnt table
NJW = KK + MCW            # 8 j-chunks in wt table
NVI = int(os.environ.get("K_NVI", "5"))   # v-tiles per W DMA (512B descriptors)
NWARM = int(os.environ.get("K_NWARM", "28"))
MMBUFS = int(os.environ.get("K_MMBUFS", "6"))
TPBUFS = int(os.environ.get("K_TPBUFS", "1"))
TPE_BUFS = int(os.environ.get("K_TPE", "1"))
TPO_BUFS = int(os.environ.get("K_TPO", "1"))
EVAC = os.environ.get("K_EVAC", "alt")    # alt | vi | dve | act
TINNER = int(os.environ.get("K_TINNER", "0"))  # share ldweights across t-chunks
TOUTER = int(os.environ.get("K_TOUTER", "1"))  # t-chunk outer within W chunk
DEDUP = int(os.environ.get("K_DEDUP", "1"))
OBATCH = int(os.environ.get("K_OBATCH", "1"))  # pair t-chunks per out DMA
WTPB = int(os.environ.get("K_WTPB", "3"))      # W-chunk prefetch depth
OUTPB = int(os.environ.get("K_OUTPB", "6"))    # out staging bufs
EPS = 1e-5
SCALE = 512.0
# (lhsT j in wt, rhs j in hnt): Whi.hhi x3, Whi.hlo x2, Wlo.hhi x1
PAIRS = [(0, 0), (2, 2), (4, 4), (0, 6), (2, 8), (6, 0)]

_cache = {}


def _dedup_ldweights(nc):
    removed = 0
    for blk in nc.m.functions[0].blocks:
        cur = None
        keep = []
        for inst in blk.instructions:
            nm = type(inst).__name__
            if nm == "InstLdweights":
                si = inst.sync_info
                clean = si is None or (
                    len(si.on_wait) == 0 and len(si.on_update) == 0
                )
                key = str(inst.ins[0])
                if clean and cur == key:
                    removed += 1
                    continue
                cur = key
            elif nm == "InstMatmult":
                if inst.is_transpose:
                    cur = None
            keep.append(inst)
        if removed:
            blk.instructions[:] = keep
    return removed


def _build():
    nc = bacc.Bacc("TRN2", target_bir_lowering=False, debug=False, num_devices=NC)
    emb = nc.dram_tensor("emb", [V, NJ * 128], f8, kind="ExternalInput")
    idx = nc.dram_tensor("idx", [128, NG], i32, kind="ExternalInput")
    ident_d = nc.dram_tensor("ident", [128, 128], f8, kind="ExternalInput")
    wt = nc.dram_tensor("wt", [128, NJW, VSH], f8, kind="ExternalInput")
    outT = nc.dram_tensor("logitsT", [VSH, TCL], bf16, kind="ExternalOutput")

    DR = mybir.MatmulPerfMode.DoubleRow

    with tile.TileContext(nc) as tc:
        with (
            tc.tile_pool(name="const", bufs=1) as constp,
            tc.tile_pool(name="hntp", bufs=1) as hntp,
            tc.tile_pool(name="wtp", bufs=WTPB) as wtp,
            tc.tile_pool(name="outp", bufs=OUTPB) as outp,
            tc.tile_pool(name="gp", bufs=4) as gp,
            tc.tile_pool(name="tps", bufs=TPBUFS, space="PSUM") as tps,
            tc.tile_pool(name="mpsum", bufs=MMBUFS, space="PSUM") as mpp,
        ):
            idxsb = constp.tile([128, NG], i32)
            nc.sync.dma_start(out=idxsb[:], in_=idx[:])
            ident = constp.tile([128, 128], f8)
            nc.sync.dma_start(out=ident[:], in_=ident_d[:])
            hnt = hntp.tile([128, NJ, TCL], f8)

            # PE warm-up on a memset tile: starts at t~0, no DMA dependency
            warm = constp.tile([128, 256], bf16)
            nc.vector.memset(warm[:], 0.25)
            wps = mpp.tile([128, 512], f32, tag="mm", name="warm_ps")
            for i in range(NWARM):
                nc.tensor.matmul(out=wps[:, 0:256], lhsT=warm[:, 0:128],
                                 rhs=warm[:], start=True, stop=True)

            # phase 1 (emitted interleaved with early phase 2): per g-tile,
            # gather 128 rows, 10 fp8 PE transposes (element step 2 into
            # even/odd 1-bank PSUM tiles), two strided copies (DVE + Act)
            # pack them into hnt.
            def emit_g(g):
                h = gp.tile([128, NJ * 128], f8, tag="h", name=f"h_{g}")
                nc.gpsimd.indirect_dma_start(
                    out=h[:], out_offset=None, in_=emb[:],
                    in_offset=IndirectOffsetOnAxis(ap=idxsb[:, g:g + 1], axis=0),
                )
                ptE = tps.tile([128, NJ // 2, 256], f8, tag="tpE",
                               bufs=TPE_BUFS, name=f"ptE_{g}")
                ptO = tps.tile([128, NJ // 2, 256], f8, tag="tpO",
                               bufs=TPO_BUFS, name=f"ptO_{g}")
                for j in range(NJ):
                    pt = ptE if j % 2 == 0 else ptO
                    nc.tensor.transpose(out=pt[:, j >> 1, ::2],
                                        in_=h[:, j * 128:(j + 1) * 128],
                                        identity=ident[:])
                nc.vector.tensor_copy(
                    out=hnt[:, 0:NJ:2, g * 128:(g + 1) * 128],
                    in_=ptE[:, :, ::2])
                nc.scalar.copy(
                    out=hnt[:, 1:NJ:2, g * 128:(g + 1) * 128],
                    in_=ptO[:, :, ::2])

            GILEAVE = int(os.environ.get("K_GILEAVE", "0"))
            for g in range(4 if GILEAVE else NG):
                emit_g(g)
            next_g = 4 if GILEAVE else NG

            # phase 2: stream W shard once, 99 v-tiles x 4 t-chunks x 7 mms.
            # W-chunk DMAs are emitted one chunk ahead of their consumers so
            # the SP queue dispatches them before the previous chunk's out
            # DMAs (kills a 2us PE stall at every chunk boundary).
            chunk_starts = list(range(0, NVT, NVI))

            def issue_w(v0):
                nvi = min(NVI, NVT - v0)
                wtt = wtp.tile([128, NJW, nvi * 128], f8, tag="wt",
                               name=f"wtt_{v0}")
                nc.sync.dma_start(out=wtt[:],
                                  in_=wt[:, :, v0 * 128:v0 * 128 + nvi * 128])
                return wtt

            nev = 0
            obs = {}
            wtt_next = issue_w(chunk_starts[0])
            for ci, v0 in enumerate(chunk_starts):
                nvi = min(NVI, NVT - v0)
                wtt = wtt_next
                if ci + 1 < len(chunk_starts):
                    wtt_next = issue_w(chunk_starts[ci + 1])
                # t-outer within the chunk: early groups only need early
                # g-tiles, so phase 2 overlaps phase 1's tail; the rest of
                # phase 1 is emitted between chunk 0's t-blocks so PE always
                # has matmul work while gathers land.
                for t in range(NTC) if TOUTER else [None]:
                 if t is not None and t > 0 and next_g < NG:
                     for g in range(next_g, min(next_g + 4, NG)):
                         emit_g(g)
                     next_g = min(next_g + 4, NG)
                 for vi in range(nvi):
                    v = v0 + vi
                    if TOUTER:
                        ps = mpp.tile([128, 512], f32, tag="mm",
                                      name=f"mm_{v}_{t}")
                        for i, (a, b) in enumerate(PAIRS):
                            nc.tensor.matmul(
                                out=ps[:],
                                lhsT=wtt[:, a:a + 2,
                                         vi * 128:(vi + 1) * 128],
                                rhs=hnt[:, b:b + 2,
                                        t * 512:(t + 1) * 512],
                                start=(i == 0),
                                stop=(i == len(PAIRS) - 1),
                                perf_mode=DR,
                            )
                        if OBATCH:
                            if t % 2 == 0:
                                ob = outp.tile([128, 2, 512], bf16,
                                               tag="ob", name=f"ob_{v}_{t}")
                                obs[v] = ob
                            else:
                                ob = obs.pop(v)
                            dst = ob[:, t % 2, :]
                        else:
                            ob = outp.tile([128, 512], bf16, tag="ob")
                            dst = ob[:]
                        use_dve = (EVAC == "dve"
                                   or (EVAC == "alt" and nev % 2 == 0)
                                   or (EVAC == "vi" and vi % 2 == 0))
                        if use_dve:
                            nc.vector.tensor_copy(out=dst, in_=ps[:])
                        else:
                            nc.scalar.copy(out=dst, in_=ps[:])
                        nev += 1
                        if OBATCH:
                            if t % 2 == 1:
                                nc.sync.dma_start(
                                    out=outT[v * 128:(v + 1) * 128,
                                             (t - 1) * 512:(t + 1) * 512],
                                    in_=ob[:])
                        else:
                            nc.sync.dma_start(
                                out=outT[v * 128:(v + 1) * 128,
                                         t * 512:(t + 1) * 512],
                                in_=ob[:])
                    elif TINNER:
                        # pair-outer order: one ldweights serves all 4
                        # t-chunk matmuls (deduped post-compile)
                        pss = [mpp.tile([128, 512], f32, tag="mm",
                                        name=f"mm_{v}_{t}")
                               for t in range(NTC)]
                        for i, (a, b) in enumerate(PAIRS):
                            for t in range(NTC):
                                nc.tensor.matmul(
                                    out=pss[t][:],
                                    lhsT=wtt[:, a:a + 2,
                                             vi * 128:(vi + 1) * 128],
                                    rhs=hnt[:, b:b + 2,
                                            t * 512:(t + 1) * 512],
                                    start=(i == 0),
                                    stop=(i == len(PAIRS) - 1),
                                    perf_mode=DR,
                                )
                        for t in range(NTC):
                            ob = outp.tile([128, 512], bf16, tag="ob")
                            if EVAC == "dve" or (EVAC == "alt" and nev % 2 == 0):
                                nc.vector.tensor_copy(out=ob[:], in_=pss[t][:])
                            else:
                                nc.scalar.copy(out=ob[:], in_=pss[t][:])
                            nev += 1
                            nc.sync.dma_start(
                                out=outT[v * 128:(v + 1) * 128,
                                         t * 512:(t + 1) * 512],
                                in_=ob[:])
                    else:
                        for t in range(NTC):
                            ps = mpp.tile([128, 512], f32, tag="mm",
                                          name=f"mm_{v}_{t}")
                            for i, (a, b) in enumerate(PAIRS):
                                nc.tensor.matmul(
                                    out=ps[:],
                                    lhsT=wtt[:, a:a + 2,
                                             vi * 128:(vi + 1) * 128],
                                    rhs=hnt[:, b:b + 2,
                                            t * 512:(t + 1) * 512],
                                    start=(i == 0),
                                    stop=(i == len(PAIRS) - 1),
                                    perf_mode=DR,
                                )
                            ob = outp.tile([128, 512], bf16, tag="ob")
                            if EVAC == "dve" or (EVAC == "alt" and nev % 2 == 0):
                                nc.vector.tensor_copy(out=ob[:], in_=ps[:])
                            else:
                                nc.scalar.copy(out=ob[:], in_=ps[:])
                            nev += 1
                            nc.sync.dma_start(
                                out=outT[v * 128:(v + 1) * 128,
                                         t * 512:(t + 1) * 512],
                                in_=ob[:])

    nc.compile()
    if DEDUP:
        nc._ldw_removed = _dedup_ldweights(nc)
    else:
        nc._ldw_removed = 0
    return nc


def _in_maps(input_sequence, embedding, final_norm, output_embedding):
    e4m3 = ml_dtypes.float8_e4m3
    idx_flat = np.asarray(input_sequence).astype(np.int32).reshape(-1)
    fn_f = np.asarray(final_norm, dtype=np.float32)
    perm = np.argsort(-fn_f, kind="stable")
    sf = np.sqrt(fn_f[perm])[None, :]

    emb_f = np.asarray(embedding, dtype=np.float32)
    rn = 1.0 / np.sqrt(np.mean(np.square(emb_f), axis=1, keepdims=True) + EPS)
    hs = (emb_f * rn)[:, perm] * sf
    h_hi = hs.astype(e4m3)
    h_lo = (hs - h_hi.astype(np.float32))[:, :MC * 128].astype(e4m3)
    emb8 = np.ascontiguousarray(
        np.concatenate([h_hi, h_lo], axis=1))          # [V, 1280] e4m3

    w_f = np.asarray(output_embedding, dtype=np.float32)[:, perm] * sf * SCALE
    ws = np.zeros((VPAD, D), dtype=np.float32)
    ws[:V] = w_f
    w_hi = ws.astype(e4m3)
    w_lo = (ws - w_hi.astype(np.float32))[:, :MCW * 128].astype(e4m3)
    whiT = w_hi.T.reshape(KK, 128, VPAD).transpose(1, 0, 2)
    wloT = w_lo.T.reshape(MCW, 128, VPAD).transpose(1, 0, 2)
    wt_full = np.concatenate([whiT, wloT], axis=1)     # [128, 10, VPAD] e4m3
    wt_sh = [np.ascontiguousarray(wt_full[:, :, s * VSH:(s + 1) * VSH])
             for s in range(VB)]

    ident_np = np.eye(128, dtype=e4m3)
    maps = []
    for c in range(NC):
        tb, vs = divmod(c, VB)
        idx_c = np.ascontiguousarray(
            idx_flat[tb * TCL:(tb + 1) * TCL].reshape(NG, 128).T)
        maps.append({"emb": emb8, "idx": idx_c, "ident": ident_np,
                     "wt": wt_sh[vs]})
    return maps


def _run(in_maps, trace=False):
    if "nc" not in _cache:
        _cache["nc"] = _build()
    return run_bass_kernel_spmd(_cache["nc"], in_maps, list(range(NC)), trace=trace)


def kernel(input_sequence, embedding, final_norm, output_embedding):
    maps = _in_maps(input_sequence, embedding, final_norm, output_embedding)
    res = _run(maps)
    full = np.empty((T, VPAD), dtype=np.float32)
    inv = np.float32(1.0 / SCALE)
    for c in range(NC):
        tb, vs = divmod(c, VB)
        full[tb * TCL:(tb + 1) * TCL, vs * VSH:(vs + 1) * VSH] = \
            res.results[c]["logitsT"].T.astype(np.float32) * inv
    return np.ascontiguousarray(full[:, :V]).reshape(B, S, V)


# revision 40
# speedup vs baseline: 1.0016x; 1.0016x over previous
"""Embedding lookup + RMSNorm + tied logits projection on 8 trn2 NeuronCores.

Strategy (2-way token x 4-way vocab, fp8 DoubleRow), v6:
  - RMSNorm folded into the embedding table on the host; final_norm split as
    sqrt(fn) into BOTH operands; contraction dims PERMUTED by descending fn.
  - Both operands quantized to e4m3 with a single power-of-2 scale (512 on W):
      W'' = W[:,perm]*sqrt(fn_sorted)*512,  h'' = hn[:,perm]*sqrt(fn_sorted)
      W_hi = e4m3(W''), W_lo = e4m3(W''-W_hi)   (lo kept for top 256 dims)
      h_hi = e4m3(h''), h_lo = e4m3(h''-h_hi)   (lo kept for top 512 dims)
    logits*512 ~= h_hi@W_hi' + h_lo@W_hi'(512) + h_hi@W_lo'(256): 6 fp8
    DoubleRow matmuls per group; measured rel err 1.53e-2 (gate 2e-2).
  - Core c owns token block c//4 (2048 tokens) and vocab shard c%4 (12672
    padded vocab rows). All matmuls are fp8 DoubleRow (0.5 cycles/row, k=256
    per instruction): 6 instructions per [128v x 512t] PSUM group.
  - Phase 1: gather 2048 rows (16 indirect DMAs of 1280B rows), PE-transpose
    to hnt [d_chunk, j, t], DVE evacuation per g-tile.
  - Phase 2: stream W shard once (10*512B descriptors per 4-v-tile chunk),
    t-chunk-outer order within each chunk (overlaps phase 1's tail), PSUM
    f32 accumulate, bf16 out, evacuations alternate DVE/Act engines, out
    DMAs cover t-chunk pairs (halves the 625ns/DMA HWDGE issue cost).
  - Host assembles: outT.T * (1/512), scatter into [T, VPAD], slice vocab.

  Measured on the 8 axon trn2 cores: rel err 1.527e-2 (gate 2e-2),
  exec 285288 ns vs 517214 ns bf16 baseline (1.81x).
"""
import os
import sys

sys.path.insert(0, "/opt/trn_rl_repo")

import numpy as np
import ml_dtypes

import concourse.mybir as mybir
import concourse.tile as tile
from concourse import bacc
from concourse.bass import IndirectOffsetOnAxis
from concourse.bass_utils import run_bass_kernel_spmd

f32 = mybir.dt.float32
bf16 = mybir.dt.bfloat16
f8 = mybir.dt.float8e4
i32 = mybir.dt.int32

B, S, V, D = 2, 2048, 50257, 768
T = B * S                 # 4096 tokens
NC = 8                    # cores
TB = 2                    # token blocks
VB = 4                    # vocab blocks
TCL = T // TB             # 2048 tokens per core
VPAD = 50688              # 4 * 12672, 12672 = 99*128
VSH = VPAD // VB          # 12672 vocab rows per core
NVT = VSH // 128          # 99 v-tiles per core
NG = TCL // 128           # 16 gather tiles
NTC = TCL // 512          # 4 moving-dim chunks
KK = D // 128             # 6 hi d-chunks
MC = 4                    # h-side lo d-chunks: top 512 dims by fn
MCW = 2                   # W-side lo d-chunks: top 256 dims by fn
NJ = KK + MC              # 10 j-chunks in emb/hnt table
NJW = KK + MCW            # 8 j-chunks in wt table
NVI = int(os.environ.get("K_NVI", "5"))   # v-tiles per W DMA (512B descriptors)
NWARM = int(os.environ.get("K_NWARM", "28"))
MMBUFS = int(os.environ.get("K_MMBUFS", "4"))
TPBUFS = int(os.environ.get("K_TPBUFS", "1"))
TPE_BUFS = int(os.environ.get("K_TPE", "2"))
TPO_BUFS = int(os.environ.get("K_TPO", "2"))
EVAC = os.environ.get("K_EVAC", "alt")    # alt | vi | dve | act
TINNER = int(os.environ.get("K_TINNER", "0"))  # share ldweights across t-chunks
TOUTER = int(os.environ.get("K_TOUTER", "1"))  # t-chunk outer within W chunk
DEDUP = int(os.environ.get("K_DEDUP", "1"))
OBATCH = int(os.environ.get("K_OBATCH", "1"))  # pair t-chunks per out DMA
WTPB = int(os.environ.get("K_WTPB", "3"))      # W-chunk prefetch depth
OUTPB = int(os.environ.get("K_OUTPB", "6"))    # out staging bufs
EPS = 1e-5
SCALE = 512.0
# (lhsT j in wt, rhs j in hnt): Whi.hhi x3, Whi.hlo x2, Wlo.hhi x1
PAIRS = [(0, 0), (2, 2), (4, 4), (0, 6), (2, 8), (6, 0)]

_cache = {}


def _dedup_ldweights(nc):
    removed = 0
    for blk in nc.m.functions[0].blocks:
        cur = None
        keep = []
        for inst in blk.instructions:
            nm = type(inst).__name__
            if nm == "InstLdweights":
                si = inst.sync_info
                clean = si is None or (
                    len(si.on_wait) == 0 and len(si.on_update) == 0
                )
                key = str(inst.ins[0])
                if clean and cur == key:
                    removed += 1
                    continue
                cur = key
            elif nm == "InstMatmult":
                if inst.is_transpose:
                    cur = None
            keep.append(inst)
        if removed:
            blk.instructions[:] = keep
    return removed


def _build():
    nc = bacc.Bacc("TRN2", target_bir_lowering=False, debug=False, num_devices=NC)
    emb = nc.dram_tensor("emb", [V, NJ * 128], f8, kind="ExternalInput")
    idx = nc.dram_tensor("idx", [128, NG], i32, kind="ExternalInput")
    ident_d = nc.dram_tensor("ident", [128, 128], f8, kind="ExternalInput")
    wt = nc.dram_tensor("wt", [128, NJW, VSH], f8, kind="ExternalInput")
    outT = nc.dram_tensor("logitsT", [VSH, TCL], bf16, kind="ExternalOutput")

    DR = mybir.MatmulPerfMode.DoubleRow

    with tile.TileContext(nc) as tc:
        with (
            tc.tile_pool(name="const", bufs=1) as constp,
            tc.tile_pool(name="hntp", bufs=1) as hntp,
            tc.tile_pool(name="wtp", bufs=WTPB) as wtp,
            tc.tile_pool(name="outp", bufs=OUTPB) as outp,
            tc.tile_pool(name="gp", bufs=4) as gp,
            tc.tile_pool(name="tps", bufs=TPBUFS, space="PSUM") as tps,
            tc.tile_pool(name="mpsum", bufs=MMBUFS, space="PSUM") as mpp,
        ):
            idxsb = constp.tile([128, NG], i32)
            nc.sync.dma_start(out=idxsb[:], in_=idx[:])
            ident = constp.tile([128, 128], f8)
            nc.sync.dma_start(out=ident[:], in_=ident_d[:])
            hnt = hntp.tile([128, NJ, TCL], f8)

            # PE warm-up on a memset tile: starts at t~0, no DMA dependency
            warm = constp.tile([128, 256], bf16)
            nc.vector.memset(warm[:], 0.25)
            wps = mpp.tile([128, 512], f32, tag="mm", name="warm_ps")
            for i in range(NWARM):
                nc.tensor.matmul(out=wps[:, 0:256], lhsT=warm[:, 0:128],
                                 rhs=warm[:], start=True, stop=True)

            # phase 1 (emitted interleaved with early phase 2): per g-tile,
            # gather 128 rows, 10 fp8 PE transposes (element step 2 into
            # even/odd 1-bank PSUM tiles), two strided copies (DVE + Act)
            # pack them into hnt.
            def emit_g(g):
                h = gp.tile([128, NJ * 128], f8, tag="h", name=f"h_{g}")
                nc.gpsimd.indirect_dma_start(
                    out=h[:], out_offset=None, in_=emb[:],
                    in_offset=IndirectOffsetOnAxis(ap=idxsb[:, g:g + 1], axis=0),
                )
                ptE = tps.tile([128, NJ // 2, 256], f8, tag="tpE",
                               bufs=TPE_BUFS, name=f"ptE_{g}")
                ptO = tps.tile([128, NJ // 2, 256], f8, tag="tpO",
                               bufs=TPO_BUFS, name=f"ptO_{g}")
                for j in range(NJ):
                    pt = ptE if j % 2 == 0 else ptO
                    nc.tensor.transpose(out=pt[:, j >> 1, ::2],
                                        in_=h[:, j * 128:(j + 1) * 128],
                                        identity=ident[:])
                nc.vector.tensor_copy(
                    out=hnt[:, 0:NJ:2, g * 128:(g + 1) * 128],
                    in_=ptE[:, :, ::2])
                nc.scalar.copy(
                    out=hnt[:, 1:NJ:2, g * 128:(g + 1) * 128],
                    in_=ptO[:, :, ::2])

            GILEAVE = int(os.environ.get("K_GILEAVE", "0"))
            for g in range(4 if GILEAVE else NG):
                emit_g(g)
            next_g = 4 if GILEAVE else NG

            # phase 2: stream W shard once, 99 v-tiles x 4 t-chunks x 7 mms.
            # W-chunk DMAs are emitted one chunk ahead of their consumers so
            # the SP queue dispatches them before the previous chunk's out
            # DMAs (kills a 2us PE stall at every chunk boundary).
            chunk_starts = list(range(0, NVT, NVI))

            def issue_w(v0):
                nvi = min(NVI, NVT - v0)
                wtt = wtp.tile([128, NJW, nvi * 128], f8, tag="wt",
                               name=f"wtt_{v0}")
                nc.sync.dma_start(out=wtt[:],
                                  in_=wt[:, :, v0 * 128:v0 * 128 + nvi * 128])
                return wtt

            nev = 0
            obs = {}
            wtt_next = issue_w(chunk_starts[0])
            for ci, v0 in enumerate(chunk_starts):
                nvi = min(NVI, NVT - v0)
                wtt = wtt_next
                if ci + 1 < len(chunk_starts):
                    wtt_next = issue_w(chunk_starts[ci + 1])
                # t-outer within the chunk: early groups only need early
                # g-tiles, so phase 2 overlaps phase 1's tail; the rest of
                # phase 1 is emitted between chunk 0's t-blocks so PE always
                # has matmul work while gathers land.
                for t in range(NTC) if TOUTER else [None]:
                 if t is not None and t > 0 and next_g < NG:
                     for g in range(next_g, min(next_g + 4, NG)):
                         emit_g(g)
                     next_g = min(next_g + 4, NG)
                 for vi in range(nvi):
                    v = v0 + vi
                    if TOUTER:
                        ps = mpp.tile([128, 512], f32, tag="mm",
                                      name=f"mm_{v}_{t}")
                        for i, (a, b) in enumerate(PAIRS):
                            nc.tensor.matmul(
                                out=ps[:],
                                lhsT=wtt[:, a:a + 2,
                                         vi * 128:(vi + 1) * 128],
                                rhs=hnt[:, b:b + 2,
                                        t * 512:(t + 1) * 512],
                                start=(i == 0),
                                stop=(i == len(PAIRS) - 1),
                                perf_mode=DR,
                            )
                        if OBATCH:
                            if t % 2 == 0:
                                ob = outp.tile([128, 2, 512], bf16,
                                               tag="ob", name=f"ob_{v}_{t}")
                                obs[v] = ob
                            else:
                                ob = obs.pop(v)
                            dst = ob[:, t % 2, :]
                        else:
                            ob = outp.tile([128, 512], bf16, tag="ob")
                            dst = ob[:]
                        use_dve = (EVAC == "dve"
                                   or (EVAC == "alt" and nev % 2 == 0)
                                   or (EVAC == "vi" and vi % 2 == 0))
                        if use_dve:
                            nc.vector.tensor_copy(out=dst, in_=ps[:])
                        else:
                            nc.scalar.copy(out=dst, in_=ps[:])
                        nev += 1
                        if OBATCH:
                            if t % 2 == 1:
                                nc.sync.dma_start(
                                    out=outT[v * 128:(v + 1) * 128,
                                             (t - 1) * 512:(t + 1) * 512],
                                    in_=ob[:])
                        else:
                            nc.sync.dma_start(
                                out=outT[v * 128:(v + 1) * 128,
                                         t * 512:(t + 1) * 512],
                                in_=ob[:])
                    elif TINNER:
                        # pair-outer order: one ldweights serves all 4
                        # t-chunk matmuls (deduped post-compile)
                        pss = [mpp.tile([128, 512], f32, tag="mm",
                                        name=f"mm_{v}_{t}")
                               for t in range(NTC)]
                        for i, (a, b) in enumerate(PAIRS):
                            for t in range(NTC):
                                nc.tensor.matmul(
                                    out=pss[t][:],
                                    lhsT=wtt[:, a:a + 2,
                                             vi * 128:(vi + 1) * 128],
                                    rhs=hnt[:, b:b + 2,
                                            t * 512:(t + 1) * 512],
                                    start=(i == 0),
                                    stop=(i == len(PAIRS) - 1),
                                    perf_mode=DR,
                                )
                        for t in range(NTC):
                            ob = outp.tile([128, 512], bf16, tag="ob")
                            if EVAC == "dve" or (EVAC == "alt" and nev % 2 == 0):
                                nc.vector.tensor_copy(out=ob[:], in_=pss[t][:])
                            else:
                                nc.scalar.copy(out=ob[:], in_=pss[t][:])
                            nev += 1
                            nc.sync.dma_start(
                                out=outT[v * 128:(v + 1) * 128,
                                         t * 512:(t + 1) * 512],
                                in_=ob[:])
                    else:
                        for t in range(NTC):
                            ps = mpp.tile([128, 512], f32, tag="mm",
                                          name=f"mm_{v}_{t}")
                            for i, (a, b) in enumerate(PAIRS):
                                nc.tensor.matmul(
                                    out=ps[:],
                                    lhsT=wtt[:, a:a + 2,
                                             vi * 128:(vi + 1) * 128],
                                    rhs=hnt[:, b:b + 2,
                                            t * 512:(t + 1) * 512],
                                    start=(i == 0),
                                    stop=(i == len(PAIRS) - 1),
                                    perf_mode=DR,
                                )
                            ob = outp.tile([128, 512], bf16, tag="ob")
                            if EVAC == "dve" or (EVAC == "alt" and nev % 2 == 0):
                                nc.vector.tensor_copy(out=ob[:], in_=ps[:])
                            else:
                                nc.scalar.copy(out=ob[:], in_=ps[:])
                            nev += 1
                            nc.sync.dma_start(
                                out=outT[v * 128:(v + 1) * 128,
                                         t * 512:(t + 1) * 512],
                                in_=ob[:])

    nc.compile()
    if DEDUP:
        nc._ldw_removed = _dedup_ldweights(nc)
    else:
        nc._ldw_removed = 0
    return nc


def _in_maps(input_sequence, embedding, final_norm, output_embedding):
    e4m3 = ml_dtypes.float8_e4m3
    idx_flat = np.asarray(input_sequence).astype(np.int32).reshape(-1)
    fn_f = np.asarray(final_norm, dtype=np.float32)
    perm = np.argsort(-fn_f, kind="stable")
    sf = np.sqrt(fn_f[perm])[None, :]

    emb_f = np.asarray(embedding, dtype=np.float32)
    rn = 1.0 / np.sqrt(np.mean(np.square(emb_f), axis=1, keepdims=True) + EPS)
    hs = (emb_f * rn)[:, perm] * sf
    h_hi = hs.astype(e4m3)
    h_lo = (hs - h_hi.astype(np.float32))[:, :MC * 128].astype(e4m3)
    emb8 = np.ascontiguousarray(
        np.concatenate([h_hi, h_lo], axis=1))          # [V, 1280] e4m3

    w_f = np.asarray(output_embedding, dtype=np.float32)[:, perm] * sf * SCALE
    ws = np.zeros((VPAD, D), dtype=np.float32)
    ws[:V] = w_f
    w_hi = ws.astype(e4m3)
    w_lo = (ws - w_hi.astype(np.float32))[:, :MCW * 128].astype(e4m3)
    whiT = w_hi.T.reshape(KK, 128, VPAD).transpose(1, 0, 2)
    wloT = w_lo.T.reshape(MCW, 128, VPAD).transpose(1, 0, 2)
    wt_full = np.concatenate([whiT, wloT], axis=1)     # [128, 10, VPAD] e4m3
    wt_sh = [np.ascontiguousarray(wt_full[:, :, s * VSH:(s + 1) * VSH])
             for s in range(VB)]

    ident_np = np.eye(128, dtype=e4m3)
    maps = []
    for c in range(NC):
        tb, vs = divmod(c, VB)
        idx_c = np.ascontiguousarray(
            idx_flat[tb * TCL:(tb + 1) * TCL].reshape(NG, 128).T)
        maps.append({"emb": emb8, "idx": idx_c, "ident": ident_np,
                     "wt": wt_sh[vs]})
    return maps


def _run(in_maps, trace=False):
    if "nc" not in _cache:
        _cache["nc"] = _build()
    return run_bass_kernel_spmd(_cache["nc"], in_maps, list(range(NC)), trace=trace)


def kernel(input_sequence, embedding, final_norm, output_embedding):
    maps = _in_maps(input_sequence, embedding, final_norm, output_embedding)
    res = _run(maps)
    full = np.empty((T, VPAD), dtype=np.float32)
    inv = np.float32(1.0 / SCALE)
    for c in range(NC):
        tb, vs = divmod(c, VB)
        full[tb * TCL:(tb + 1) * TCL, vs * VSH:(vs + 1) * VSH] = \
            res.results[c]["logitsT"].T.astype(np.float32) * inv
    return np.ascontiguousarray(full[:, :V]).reshape(B, S, V)
